# revision 13
# baseline (speedup 1.0000x reference)
"""BotRGCN forward on 8 TRN2 NeuronCores (Bass/Tile SPMD kernel).

Strategy (self-contained; shapes hardcoded for nn_BotRGCN1):
  - Nodes sharded 8-way (6272/core, N padded 50000->50176).
  - Dense MLPs node-parallel, feature-major on-chip ([128 feat, nodes]).
  - RGCN layer: aggregate-then-transform.  Edge messages are gathered
    with dma_gather (bf16 node features, 256B rows) and segment-summed
    on the TensorEngine via per-block weighted one-hot matmuls
    (Ind[tok,dst] = (iota==dst_local)*(1/deg)), accumulated in PSUM per
    (relation, dst-tile-of-512).  Transform = W_r^T @ acc on PE.
  - Boundary exchange: AllGather of bf16 activations between layers.
  - Edges are partitioned by dst core, grouped by (src-half, dst-tile,
    relation) so every dma_gather has an int16-safe base; the block
    schedule is the max over cores (SPMD: one program for all 8).
"""
import numpy as np
import ml_dtypes

N = 50000
M = 8
L = 6272            # nodes per core (N padded to 50176)
NPAD = M * L
D = 128
DDES = 768
R = 5
TW = 512            # dst tile width
NT = 13             # ceil(L/TW): 12 full + 1 of 128
SPLIT = 32768       # src half split for int16 gather indices
CHUNK = 128        # tokens per dma_gather (64 desc/engine packet limit)
SLOPE = 0.01
NQUEUES = 4

_LAST = {}          # exec stats for test harness


def _tile_w(t):
    return min(TW, L - t * TW)


def _prep_edges(edge_index, edge_type):
    """Per-core token streams + shared (max-over-cores) block schedule."""
    src = np.asarray(edge_index[0], dtype=np.int64)
    dst = np.asarray(edge_index[1], dtype=np.int64)
    et = np.asarray(edge_type, dtype=np.int64)

    core = dst // L
    dloc = dst % L

    # group tokens per core: key = (half, tile, rel)
    per_core_groups = []
    for m in range(M):
        sel = np.nonzero(core == m)[0]
        s, dl, r = src[sel], dloc[sel], et[sel]
        cnt = np.zeros((R, L), np.float32)
        np.add.at(cnt, (r, dl), 1.0)
        rcp = 1.0 / np.maximum(cnt, 1.0)
        h = (s >= SPLIT).astype(np.int64)
        t = dl // TW
        key = ((h * NT) + t) * R + r
        order = np.argsort(key, kind="stable")
        s, dl, r, h, t, key = s[order], dl[order], r[order], h[order], t[order], key[order]
        groups = {}
        bounds = np.searchsorted(key, np.arange(2 * NT * R + 1))
        for gk in range(2 * NT * R):
            a, b = bounds[gk], bounds[gk + 1]
            gh, gt, gr = gk // (NT * R), (gk // R) % NT, gk % R
            gs = s[a:b]
            gidx = np.where(gh == 0, gs, gs - SPLIT).astype(np.int16)
            gdst = (dl[a:b] - gt * TW).astype(np.float32)
            grcp = rcp[r[a:b], dl[a:b]].astype(np.float32)
            groups[(gh, gt, gr)] = (gidx, gdst, grcp)
        per_core_groups.append(groups)

    # shared schedule: blocks per group = max over cores (>=1)
    nblk = {}
    for gh in range(2):
        for gt in range(NT):
            for gr in range(R):
                mx = max(len(per_core_groups[m][(gh, gt, gr)][0]) for m in range(M))
                nblk[(gh, gt, gr)] = max(1, -(-mx // 128))

    # build padded per-core streams in fixed (h, t, r) order
    TTOT = 128 * sum(nblk.values())
    NBLK = TTOT // 128
    gidx_all = np.zeros((M, TTOT), np.int16)
    gdst_all = np.full((M, TTOT), -1.0, np.float32)
    grcp_all = np.zeros((M, TTOT), np.float32)
    pos = 0
    sched = []   # per block: (h, t, r, first, last)
    half_tok = [0, 0]
    for gh in range(2):
        for gt in range(NT):
            for gr in range(R):
                nb = nblk[(gh, gt, gr)]
                for m in range(M):
                    gi, gd, gc = per_core_groups[m][(gh, gt, gr)]
                    n = len(gi)
                    gidx_all[m, pos:pos + n] = gi
                    gdst_all[m, pos:pos + n] = gd
                    grcp_all[m, pos:pos + n] = gc
                for j in range(nb):
                    sched.append((gh, gt, gr, j == 0, j == nb - 1))
                pos += nb * 128
                half_tok[gh] += nb * 128
    assert pos == TTOT

    # gather chunks: per half, cut every CHUNK tokens
    chunks = []  # (half, tok_start, ntok)
    off = 0
    for gh in range(2):
        th = half_tok[gh]
        s0 = 0
        while s0 < th:
            n = min(CHUNK, th - s0)
            chunks.append((gh, off + s0, n))
            s0 += n
        off += th

    # wrap layouts for SBUF
    gidx_w = np.tile(
        gidx_all.reshape(M, TTOT // 16, 16).transpose(0, 2, 1), (1, 8, 1)
    ).copy()                                            # [M, 128, TTOT//16]
    gdst_w = gdst_all.reshape(M, NBLK, 128).transpose(0, 2, 1).copy()  # [M,128,NBLK]
    grcp_w = grcp_all.reshape(M, NBLK, 128).transpose(0, 2, 1).copy()
    return gidx_w, gdst_w, grcp_w, sched, chunks, TTOT, NBLK


def _build(sched, chunks, TTOT, NBLK):
    from concourse import bacc, tile, mybir

    nc = bacc.Bacc("TRN2", target_bir_lowering=False, debug=False,
                   num_devices=M, num_swdge_queues=NQUEUES)
    f32, bf16, i16 = mybir.dt.float32, mybir.dt.bfloat16, mybir.dt.int16
    f16 = mybir.dt.float16
    Alu = mybir.AluOpType

    desT_d = nc.dram_tensor("desT", [DDES, L], f32, kind="ExternalInput")
    gidx_d = nc.dram_tensor("gidx", [128, TTOT // 16], i16, kind="ExternalInput")
    gdst_d = nc.dram_tensor("gdst", [128, NBLK], f32, kind="ExternalInput")
    grcp_d = nc.dram_tensor("grcp", [128, NBLK], f32, kind="ExternalInput")
    wdes_d = nc.dram_tensor("wdes", [DDES, D], f32, kind="ExternalInput")
    win_d = nc.dram_tensor("win", [D, D], f32, kind="ExternalInput")
    wroot_d = nc.dram_tensor("wroot", [D, D], f32, kind="ExternalInput")
    wrel_d = nc.dram_tensor("wrel", [R, D, D], f32, kind="ExternalInput")
    wout1_d = nc.dram_tensor("wout1", [D, D], f32, kind="ExternalInput")
    wout2_d = nc.dram_tensor("wout2", [D, 2], f32, kind="ExternalInput")
    bias_d = nc.dram_tensor("bias", [D, 4], f32, kind="ExternalInput")  # des,in,rgcn,out1
    bout2_d = nc.dram_tensor("bout2", [2, 1], f32, kind="ExternalInput")
    out_d = nc.dram_tensor("out", [2, L], f32, kind="ExternalOutput")

    y_loc = nc.dram_tensor("y_loc", [L, D], f16)
    xf = [nc.dram_tensor(f"xf{i}", [NPAD, D], f16, addr_space="Shared")
          for i in range(2)]

    iota = nc.inline_tensor(
        np.broadcast_to(np.arange(TW, dtype=np.float16), (128, TW)).copy(), "iota")
    ident = nc.inline_tensor(np.eye(128, dtype=np.float32), "ident")

    NCH = -(-L // TW)  # mlp col chunks (= NT)

    with tile.TileContext(nc) as tc:
        with (
            tc.tile_pool(name="cst", bufs=1) as cst,
            tc.tile_pool(name="big", bufs=2) as big,
            tc.tile_pool(name="wk", bufs=4) as wk,
            tc.tile_pool(name="ps", bufs=1, space="PSUM") as psp,
        ):
            # ---- constants to SBUF ----
            iota_sb = cst.tile([128, TW], f16)
            nc.sync.dma_start(out=iota_sb[:], in_=iota[:])
            ident_sb = cst.tile([128, 128], f32)
            nc.sync.dma_start(out=ident_sb[:], in_=ident[:])
            gidx_sb = cst.tile([128, TTOT // 16], i16)
            nc.sync.dma_start(out=gidx_sb[:], in_=gidx_d[:])
            gdst_sb = cst.tile([128, NBLK], f32)
            nc.sync.dma_start(out=gdst_sb[:], in_=gdst_d[:])
            grcp_sb = cst.tile([128, NBLK], f32)
            nc.sync.dma_start(out=grcp_sb[:], in_=grcp_d[:])
            wdes_sb = cst.tile([128, 6, D], f32)
            for k in range(6):
                nc.sync.dma_start(out=wdes_sb[:, k, :], in_=wdes_d[k * 128:(k + 1) * 128, :])
            win_sb = cst.tile([128, D], f32)
            nc.sync.dma_start(out=win_sb[:], in_=win_d[:])
            wroot_sb = cst.tile([128, D], f32)
            nc.sync.dma_start(out=wroot_sb[:], in_=wroot_d[:])
            wrel_sb = cst.tile([128, R, D], f32)
            for r in range(R):
                nc.sync.dma_start(out=wrel_sb[:, r, :], in_=wrel_d[r])
            wout1_sb = cst.tile([128, D], f32)
            nc.sync.dma_start(out=wout1_sb[:], in_=wout1_d[:])
            wout2_sb = cst.tile([128, 2], f32)
            nc.sync.dma_start(out=wout2_sb[:], in_=wout2_d[:])
            bias_sb = cst.tile([128, 4], f32)
            nc.sync.dma_start(out=bias_sb[:], in_=bias_d[:])
            bout2_sb = cst.tile([2, 1], f32)
            nc.sync.dma_start(out=bout2_sb[:], in_=bout2_d[:])

            def leaky_from_psum(dst_f32, ps_ap, bias_col, w):
                """dst = leaky(ps + bias), feature-major [128, w]."""
                t1 = wk.tile([128, TW], f32, tag="lk1")
                nc.vector.tensor_scalar(
                    out=t1[:, :w], in0=ps_ap, scalar1=bias_col, scalar2=None,
                    op0=Alu.add)
                t2 = wk.tile([128, TW], f32, tag="lk2")
                nc.vector.tensor_scalar_mul(t2[:, :w], t1[:, :w], SLOPE)
                nc.vector.tensor_tensor(
                    out=dst_f32, in0=t1[:, :w], in1=t2[:, :w], op=Alu.max)

            def transpose_store(src_f32_ap, t, w):
                """feature-major [128, w] slice of tile t -> y_loc + (later AG)."""
                for b in range(-(-w // 128)):
                    bw = min(128, w - b * 128)
                    trp = psp.tile([128, 128], f32, tag="tr")
                    nc.tensor.transpose(
                        trp[:bw, :], src_f32_ap[:, b * 128:b * 128 + bw], ident_sb[:])
                    ynm = wk.tile([128, D], f16, tag="ynm")
                    nc.vector.tensor_copy(ynm[:bw, :], trp[:bw, :])
                    r0 = t * TW + b * 128
                    nc.sync.dma_start(out=y_loc[r0:r0 + bw, :], in_=ynm[:bw, :])

            # ================= MLP =================
            x1T = big.tile([128, L], f32, tag="bigT")
            for c in range(NCH):
                w = _tile_w(c)
                ps = psp.tile([128, TW], f32, tag="out")
                for k in range(6):
                    dt = wk.tile([128, TW], f32, tag="des")
                    nc.sync.dma_start(
                        out=dt[:, :w],
                        in_=desT_d[k * 128:(k + 1) * 128, c * TW:c * TW + w])
                    nc.tensor.matmul(ps[:, :w], wdes_sb[:, k, :], dt[:, :w],
                                     start=(k == 0), stop=(k == 5))
                x0c = wk.tile([128, TW], f32, tag="x0c")
                leaky_from_psum(x0c[:, :w], ps[:, :w], bias_sb[:, 0:1], w)
                ps2 = psp.tile([128, TW], f32, tag="out")
                nc.tensor.matmul(ps2[:, :w], win_sb[:], x0c[:, :w],
                                 start=True, stop=True)
                leaky_from_psum(x1T[:, c * TW:c * TW + w], ps2[:, :w],
                                bias_sb[:, 1:2], w)
                transpose_store(x1T[:, c * TW:c * TW + w], c, w)

            nc.gpsimd.collective_compute(
                "AllGather", mybir.AluOpType.bypass,
                replica_groups=[list(range(M))],
                ins=[y_loc[:]], outs=[xf[0][:]])

            # ================= RGCN layers =================
            # queue must track Tile's global DMASW lane rotation: gathers are
            # the ONLY Pool-engine DMA insts, so a single global counter keeps
            # lane%4 == queue for every gather.
            qctr = [0]

            def rgcn_layer(xfull, x_curT, is_last):
                yT = big.tile([128, L], f32, tag="bigT")
                half_base = [xfull[0:SPLIT, :], xfull[SPLIT:NPAD, :]]
                agg = {}       # r -> psum tile for current (h,t)
                accTs = {}     # r -> drained SBUF acc for current (h,t)

                def finish_tile(h, t):
                    w = _tile_w(t)
                    ops = psp.tile([128, TW], f32, tag="out")
                    if h == 0:
                        nc.tensor.matmul(ops[:, :w], wroot_sb[:],
                                         x_curT[:, t * TW:t * TW + w],
                                         start=True, stop=False)
                    for ri in range(R):
                        nc.tensor.matmul(ops[:, :w], wrel_sb[:, ri, :],
                                         accTs[ri][:, :w],
                                         start=(h == 1 and ri == 0),
                                         stop=(ri == R - 1))
                    if h == 0:
                        nc.vector.tensor_scalar(
                            out=yT[:, t * TW:t * TW + w], in0=ops[:, :w],
                            scalar1=bias_sb[:, 2:3], scalar2=None, op0=Alu.add)
                    else:
                        nc.vector.tensor_tensor(
                            out=yT[:, t * TW:t * TW + w], in0=ops[:, :w],
                            in1=yT[:, t * TW:t * TW + w], op=Alu.add)
                        if not is_last:
                            transpose_store(yT[:, t * TW:t * TW + w], t, w)
                    accTs.clear()

                blk_i = 0
                cur = None  # (h, t)
                for (h, s0, ntok) in chunks:
                    nb = ntok // 128
                    g = wk.tile([128, CHUNK // 128, D], f16, tag="g")
                    nc.gpsimd.dma_gather(
                        out_ap=g[:, :nb, :],
                        in_ap=half_base[h],
                        idxs_ap=gidx_sb[:, s0 // 16:(s0 + ntok) // 16],
                        num_idxs=ntok,
                        num_idxs_reg=ntok,
                        elem_size=D,
                        queue_num=qctr[0] % NQUEUES,
                    )
                    qctr[0] += 1
                    for j in range(nb):
                        bh, bt, br, first, last = sched[blk_i]
                        assert bh == h
                        if cur is None:
                            cur = (bh, bt)
                        elif cur != (bh, bt):
                            finish_tile(*cur)
                            cur = (bh, bt)
                        w = _tile_w(bt)
                        col = s0 // 128 + j
                        nc.scalar.activation(
                            g[:, j, :], g[:, j, :],
                            mybir.ActivationFunctionType.Copy,
                            scale=grcp_sb[:, col:col + 1])
                        ind = wk.tile([128, TW], f16, tag="ind")
                        nc.vector.tensor_scalar(
                            out=ind[:, :w], in0=iota_sb[:, :w],
                            scalar1=gdst_sb[:, col:col + 1], scalar2=None,
                            op0=Alu.is_equal)
                        if first:
                            agg[br] = psp.tile([128, TW], f32, tag=f"agg{br}",
                                               name=f"agg{br}")
                        nc.tensor.matmul(agg[br][:, :w], g[:, j, :], ind[:, :w],
                                         start=first, stop=last)
                        if last:
                            acc = wk.tile([128, TW], f32, tag="accT")
                            nc.vector.tensor_copy(acc[:, :w], agg[br][:, :w])
                            accTs[br] = acc
                        blk_i += 1
                finish_tile(*cur)
                assert blk_i == len(sched)
                return yT

            y1T = rgcn_layer(xf[0], x1T, is_last=False)
            nc.gpsimd.collective_compute(
                "AllGather", mybir.AluOpType.bypass,
                replica_groups=[list(range(M))],
                ins=[y_loc[:]], outs=[xf[1][:]])
            y2T = rgcn_layer(xf[1], y1T, is_last=True)

            # ================= out MLP =================
            outT = big.tile([2, L], f32, tag="outT")
            for c in range(NCH):
                w = _tile_w(c)
                ps = psp.tile([128, TW], f32, tag="out")
                nc.tensor.matmul(ps[:, :w], wout1_sb[:],
                                 y2T[:, c * TW:c * TW + w], start=True, stop=True)
                z1 = wk.tile([128, TW], f32, tag="x0c")
                leaky_from_psum(z1[:, :w], ps[:, :w], bias_sb[:, 3:4], w)
                ps2 = psp.tile([2, TW], f32, tag="out2")
                nc.tensor.matmul(ps2[:, :w], wout2_sb[:], z1[:, :w],
                                 start=True, stop=True)
                nc.vector.tensor_scalar(
                    out=outT[:, c * TW:c * TW + w], in0=ps2[:, :w],
                    scalar1=bout2_sb[:, 0:1], scalar2=None, op0=Alu.add)
            nc.sync.dma_start(out=out_d[:], in_=outT[:])

    nc.compile()
    return nc


def kernel(des, tweet, num_prop, cat_prop, edge_index, edge_type,
           W_des, b_des, W_in, b_in, W_rel, W_root, b_rgcn,
           W_out1, b_out1, W_out2, b_out2):
    import time
    from concourse.bass_utils import run_bass_kernel_spmd

    des = np.asarray(des, np.float32)
    gidx_w, gdst_w, grcp_w, sched, chunks, TTOT, NBLK = _prep_edges(
        np.asarray(edge_index), np.asarray(edge_type))

    t0 = time.time()
    nc = _build(sched, chunks, TTOT, NBLK)
    t1 = time.time()

    des_pad = np.zeros((NPAD, DDES), np.float32)
    des_pad[:N] = des
    bias = np.stack([np.asarray(b_des, np.float32),
                     np.asarray(b_in, np.float32),
                     np.asarray(b_rgcn, np.float32),
                     np.asarray(b_out1, np.float32)], axis=1)  # [128,4]
    common = {
        "wdes": np.asarray(W_des, np.float32),
        "win": np.asarray(W_in, np.float32),
        "wroot": np.asarray(W_root, np.float32),
        "wrel": np.asarray(W_rel, np.float32),
        "wout1": np.asarray(W_out1, np.float32),
        "wout2": np.asarray(W_out2, np.float32),
        "bias": bias,
        "bout2": np.asarray(b_out2, np.float32).reshape(2, 1),
    }
    in_maps = []
    for m in range(M):
        in_maps.append({
            "desT": np.ascontiguousarray(des_pad[m * L:(m + 1) * L].T),
            "gidx": gidx_w[m], "gdst": gdst_w[m], "grcp": grcp_w[m],
            **common,
        })

    trace = bool(_LAST.get("trace"))
    res = run_bass_kernel_spmd(nc, in_maps, list(range(M)), trace=trace)
    t2 = time.time()
    _LAST["build_s"] = t1 - t0
    _LAST["run_s"] = t2 - t1
    _LAST["exec_ns"] = res.exec_time_ns
    _LAST["ttot"] = TTOT

    out = np.concatenate([res.results[m]["out"].T for m in range(M)], axis=0)
    return np.ascontiguousarray(out[:N])


# revision 14
# speedup vs baseline: 1.2701x; 1.2701x over previous
"""BotRGCN forward on 8 TRN2 NeuronCores (Bass/Tile SPMD kernel).

Strategy (self-contained; shapes hardcoded for nn_BotRGCN1):
  - Nodes sharded 8-way (6272/core, N padded 50000->50176).
  - Dense MLPs node-parallel, feature-major on-chip ([128 feat, nodes]).
  - RGCN layer: aggregate-then-transform.  Edge messages are gathered
    with dma_gather (bf16 node features, 256B rows) and segment-summed
    on the TensorEngine via per-block weighted one-hot matmuls
    (Ind[tok,dst] = (iota==dst_local)*(1/deg)), accumulated in PSUM per
    (relation, dst-tile-of-512).  Transform = W_r^T @ acc on PE.
  - Boundary exchange: AllGather of bf16 activations between layers.
  - Edges are partitioned by dst core, grouped by (src-half, dst-tile,
    relation) so every dma_gather has an int16-safe base; the block
    schedule is the max over cores (SPMD: one program for all 8).
"""
import numpy as np
import ml_dtypes

N = 50000
M = 8
L = 6272            # nodes per core (N padded to 50176)
NPAD = M * L
D = 128
DDES = 768
R = 5
TW = 512            # dst tile width
NT = 13             # ceil(L/TW): 12 full + 1 of 128
SPLIT = 32768       # src half split for int16 gather indices
CHUNK = 256        # tokens per dma_gather (64 desc/engine packet limit)
SLOPE = 0.01
NQUEUES = 4

_LAST = {}          # exec stats for test harness


def _tile_w(t):
    return min(TW, L - t * TW)


def _prep_edges(edge_index, edge_type):
    """Per-core token streams + shared (max-over-cores) block schedule."""
    src = np.asarray(edge_index[0], dtype=np.int64)
    dst = np.asarray(edge_index[1], dtype=np.int64)
    et = np.asarray(edge_type, dtype=np.int64)

    core = dst // L
    dloc = dst % L

    # group tokens per core: key = (half, tile, rel)
    per_core_groups = []
    for m in range(M):
        sel = np.nonzero(core == m)[0]
        s, dl, r = src[sel], dloc[sel], et[sel]
        cnt = np.zeros((R, L), np.float32)
        np.add.at(cnt, (r, dl), 1.0)
        rcp = 1.0 / np.maximum(cnt, 1.0)
        h = (s >= SPLIT).astype(np.int64)
        t = dl // TW
        key = ((h * NT) + t) * R + r
        order = np.argsort(key, kind="stable")
        s, dl, r, h, t, key = s[order], dl[order], r[order], h[order], t[order], key[order]
        groups = {}
        bounds = np.searchsorted(key, np.arange(2 * NT * R + 1))
        for gk in range(2 * NT * R):
            a, b = bounds[gk], bounds[gk + 1]
            gh, gt, gr = gk // (NT * R), (gk // R) % NT, gk % R
            gs = s[a:b]
            gidx = np.where(gh == 0, gs, gs - SPLIT).astype(np.int16)
            gdst = (dl[a:b] - gt * TW).astype(np.float32)
            grcp = rcp[r[a:b], dl[a:b]].astype(np.float32)
            groups[(gh, gt, gr)] = (gidx, gdst, grcp)
        per_core_groups.append(groups)

    # shared schedule: blocks per group = max over cores (>=1)
    nblk = {}
    for gh in range(2):
        for gt in range(NT):
            for gr in range(R):
                mx = max(len(per_core_groups[m][(gh, gt, gr)][0]) for m in range(M))
                nblk[(gh, gt, gr)] = max(1, -(-mx // 128))

    # build padded per-core streams in fixed (h, t, r) order
    TTOT = 128 * sum(nblk.values())
    NBLK = TTOT // 128
    gidx_all = np.zeros((M, TTOT), np.int16)
    gdst_all = np.full((M, TTOT), -1.0, np.float32)
    grcp_all = np.zeros((M, TTOT), np.float32)
    pos = 0
    sched = []   # per block: (h, t, r, first, last)
    half_tok = [0, 0]
    for gh in range(2):
        for gt in range(NT):
            for gr in range(R):
                nb = nblk[(gh, gt, gr)]
                for m in range(M):
                    gi, gd, gc = per_core_groups[m][(gh, gt, gr)]
                    n = len(gi)
                    gidx_all[m, pos:pos + n] = gi
                    gdst_all[m, pos:pos + n] = gd
                    grcp_all[m, pos:pos + n] = gc
                for j in range(nb):
                    sched.append((gh, gt, gr, j == 0, j == nb - 1))
                pos += nb * 128
                half_tok[gh] += nb * 128
    assert pos == TTOT

    # gather chunks: per half, cut every CHUNK tokens
    chunks = []  # (half, tok_start, ntok)
    off = 0
    for gh in range(2):
        th = half_tok[gh]
        s0 = 0
        while s0 < th:
            n = min(CHUNK, th - s0)
            chunks.append((gh, off + s0, n))
            s0 += n
        off += th

    # wrap layouts for SBUF
    gidx_w = np.tile(
        gidx_all.reshape(M, TTOT // 16, 16).transpose(0, 2, 1), (1, 8, 1)
    ).copy()                                            # [M, 128, TTOT//16]
    gdst_w = gdst_all.reshape(M, NBLK, 128).transpose(0, 2, 1).copy()  # [M,128,NBLK]
    grcp_w = grcp_all.reshape(M, NBLK, 128).transpose(0, 2, 1).copy()
    return gidx_w, gdst_w, grcp_w, sched, chunks, TTOT, NBLK


def _build(sched, chunks, TTOT, NBLK):
    from concourse import bacc, tile, mybir

    nc = bacc.Bacc("TRN2", target_bir_lowering=False, debug=False,
                   num_devices=M, num_swdge_queues=NQUEUES)
    f32, bf16, i16 = mybir.dt.float32, mybir.dt.bfloat16, mybir.dt.int16
    f16 = mybir.dt.float16
    Alu = mybir.AluOpType

    desT_d = nc.dram_tensor("desT", [DDES, L], f32, kind="ExternalInput")
    gidx_d = nc.dram_tensor("gidx", [128, TTOT // 16], i16, kind="ExternalInput")
    gdst_d = nc.dram_tensor("gdst", [128, NBLK], f32, kind="ExternalInput")
    grcp_d = nc.dram_tensor("grcp", [128, NBLK], f32, kind="ExternalInput")
    wdes_d = nc.dram_tensor("wdes", [DDES, D], f32, kind="ExternalInput")
    win_d = nc.dram_tensor("win", [D, D], f32, kind="ExternalInput")
    wroot_d = nc.dram_tensor("wroot", [D, D], f32, kind="ExternalInput")
    wrel_d = nc.dram_tensor("wrel", [R, D, D], f32, kind="ExternalInput")
    wout1_d = nc.dram_tensor("wout1", [D, D], f32, kind="ExternalInput")
    wout2_d = nc.dram_tensor("wout2", [D, 2], f32, kind="ExternalInput")
    bias_d = nc.dram_tensor("bias", [D, 4], f32, kind="ExternalInput")  # des,in,rgcn,out1
    bout2_d = nc.dram_tensor("bout2", [2, 1], f32, kind="ExternalInput")
    out_d = nc.dram_tensor("out", [2, L], f32, kind="ExternalOutput")

    y_loc = nc.dram_tensor("y_loc", [L, D], f16)
    xf = [nc.dram_tensor(f"xf{i}", [NPAD, D], f16, addr_space="Shared")
          for i in range(2)]

    iota = nc.inline_tensor(
        np.broadcast_to(np.arange(TW, dtype=np.float16), (128, TW)).copy(), "iota")
    ident = nc.inline_tensor(np.eye(128, dtype=np.float32), "ident")

    NCH = -(-L // TW)  # mlp col chunks (= NT)

    with tile.TileContext(nc) as tc:
        with (
            tc.tile_pool(name="cst", bufs=1) as cst,
            tc.tile_pool(name="big", bufs=2) as big,
            tc.tile_pool(name="wk", bufs=4) as wk,
            tc.tile_pool(name="ps", bufs=1, space="PSUM") as psp,
        ):
            # ---- constants to SBUF ----
            iota_sb = cst.tile([128, TW], f16)
            nc.sync.dma_start(out=iota_sb[:], in_=iota[:])
            ident_sb = cst.tile([128, 128], f32)
            nc.sync.dma_start(out=ident_sb[:], in_=ident[:])
            gidx_sb = cst.tile([128, TTOT // 16], i16)
            nc.sync.dma_start(out=gidx_sb[:], in_=gidx_d[:])
            gdst_sb = cst.tile([128, NBLK], f32)
            nc.sync.dma_start(out=gdst_sb[:], in_=gdst_d[:])
            grcp_sb = cst.tile([128, NBLK], f32)
            nc.sync.dma_start(out=grcp_sb[:], in_=grcp_d[:])
            wdes_sb = cst.tile([128, 6, D], f32)
            for k in range(6):
                nc.sync.dma_start(out=wdes_sb[:, k, :], in_=wdes_d[k * 128:(k + 1) * 128, :])
            win_sb = cst.tile([128, D], f32)
            nc.sync.dma_start(out=win_sb[:], in_=win_d[:])
            wroot_sb = cst.tile([128, D], f32)
            nc.sync.dma_start(out=wroot_sb[:], in_=wroot_d[:])
            wrel_sb = cst.tile([128, R, D], f32)
            for r in range(R):
                nc.sync.dma_start(out=wrel_sb[:, r, :], in_=wrel_d[r])
            wout1_sb = cst.tile([128, D], f32)
            nc.sync.dma_start(out=wout1_sb[:], in_=wout1_d[:])
            wout2_sb = cst.tile([128, 2], f32)
            nc.sync.dma_start(out=wout2_sb[:], in_=wout2_d[:])
            bias_sb = cst.tile([128, 4], f32)
            nc.sync.dma_start(out=bias_sb[:], in_=bias_d[:])
            bout2_sb = cst.tile([2, 1], f32)
            nc.sync.dma_start(out=bout2_sb[:], in_=bout2_d[:])

            def leaky_from_psum(dst_f32, ps_ap, bias_col, w):
                """dst = leaky(ps + bias), feature-major [128, w]."""
                t1 = wk.tile([128, TW], f32, tag="lk1")
                nc.vector.tensor_scalar(
                    out=t1[:, :w], in0=ps_ap, scalar1=bias_col, scalar2=None,
                    op0=Alu.add)
                t2 = wk.tile([128, TW], f32, tag="lk2")
                nc.vector.tensor_scalar_mul(t2[:, :w], t1[:, :w], SLOPE)
                nc.vector.tensor_tensor(
                    out=dst_f32, in0=t1[:, :w], in1=t2[:, :w], op=Alu.max)

            def transpose_store(src_f32_ap, t, w):
                """feature-major [128, w] slice of tile t -> y_loc + (later AG)."""
                for b in range(-(-w // 128)):
                    bw = min(128, w - b * 128)
                    trp = psp.tile([128, 128], f32, tag="tr")
                    nc.tensor.transpose(
                        trp[:bw, :], src_f32_ap[:, b * 128:b * 128 + bw], ident_sb[:])
                    ynm = wk.tile([128, D], f16, tag="ynm")
                    nc.vector.tensor_copy(ynm[:bw, :], trp[:bw, :])
                    r0 = t * TW + b * 128
                    nc.sync.dma_start(out=y_loc[r0:r0 + bw, :], in_=ynm[:bw, :])

            # ================= MLP =================
            x1T = big.tile([128, L], f32, tag="bigT")
            for c in range(NCH):
                w = _tile_w(c)
                ps = psp.tile([128, TW], f32, tag="out")
                for k in range(6):
                    dt = wk.tile([128, TW], f32, tag="des")
                    nc.sync.dma_start(
                        out=dt[:, :w],
                        in_=desT_d[k * 128:(k + 1) * 128, c * TW:c * TW + w])
                    nc.tensor.matmul(ps[:, :w], wdes_sb[:, k, :], dt[:, :w],
                                     start=(k == 0), stop=(k == 5))
                x0c = wk.tile([128, TW], f32, tag="x0c")
                leaky_from_psum(x0c[:, :w], ps[:, :w], bias_sb[:, 0:1], w)
                ps2 = psp.tile([128, TW], f32, tag="out")
                nc.tensor.matmul(ps2[:, :w], win_sb[:], x0c[:, :w],
                                 start=True, stop=True)
                leaky_from_psum(x1T[:, c * TW:c * TW + w], ps2[:, :w],
                                bias_sb[:, 1:2], w)
                transpose_store(x1T[:, c * TW:c * TW + w], c, w)

            nc.gpsimd.collective_compute(
                "AllGather", mybir.AluOpType.bypass,
                replica_groups=[list(range(M))],
                ins=[y_loc[:]], outs=[xf[0][:]])

            # ================= RGCN layers =================
            # queue must track Tile's global DMASW lane rotation: gathers are
            # the ONLY Pool-engine DMA insts, so a single global counter keeps
            # lane%4 == queue for every gather.
            qctr = [0]

            def rgcn_layer(xfull, x_curT, is_last):
                yT = big.tile([128, L], f32, tag="bigT")
                half_base = [xfull[0:SPLIT, :], xfull[SPLIT:NPAD, :]]
                agg = {}       # r -> psum tile for current (h,t)
                accTs = {}     # r -> drained SBUF acc for current (h,t)

                def finish_tile(h, t):
                    w = _tile_w(t)
                    ops = psp.tile([128, TW], f32, tag="out")
                    if h == 0:
                        nc.tensor.matmul(ops[:, :w], wroot_sb[:],
                                         x_curT[:, t * TW:t * TW + w],
                                         start=True, stop=False)
                    for ri in range(R):
                        nc.tensor.matmul(ops[:, :w], wrel_sb[:, ri, :],
                                         accTs[ri][:, :w],
                                         start=(h == 1 and ri == 0),
                                         stop=(ri == R - 1))
                    if h == 0:
                        nc.vector.tensor_scalar(
                            out=yT[:, t * TW:t * TW + w], in0=ops[:, :w],
                            scalar1=bias_sb[:, 2:3], scalar2=None, op0=Alu.add)
                    else:
                        nc.vector.tensor_tensor(
                            out=yT[:, t * TW:t * TW + w], in0=ops[:, :w],
                            in1=yT[:, t * TW:t * TW + w], op=Alu.add)
                        if not is_last:
                            transpose_store(yT[:, t * TW:t * TW + w], t, w)
                    accTs.clear()

                blk_i = 0
                cur = None  # (h, t)
                for (h, s0, ntok) in chunks:
                    nb = ntok // 128
                    g = wk.tile([128, CHUNK // 128, D], f16, tag="g")
                    nc.gpsimd.dma_gather(
                        out_ap=g[:, :nb, :],
                        in_ap=half_base[h],
                        idxs_ap=gidx_sb[:, s0 // 16:(s0 + ntok) // 16],
                        num_idxs=ntok,
                        num_idxs_reg=ntok,
                        elem_size=D,
                        queue_num=qctr[0] % NQUEUES,
                    )
                    qctr[0] += 1
                    for j in range(nb):
                        bh, bt, br, first, last = sched[blk_i]
                        assert bh == h
                        if cur is None:
                            cur = (bh, bt)
                        elif cur != (bh, bt):
                            finish_tile(*cur)
                            cur = (bh, bt)
                        w = _tile_w(bt)
                        col = s0 // 128 + j
                        nc.scalar.activation(
                            g[:, j, :], g[:, j, :],
                            mybir.ActivationFunctionType.Copy,
                            scale=grcp_sb[:, col:col + 1])
                        ind = wk.tile([128, TW], f16, tag="ind")
                        nc.vector.tensor_scalar(
                            out=ind[:, :w], in0=iota_sb[:, :w],
                            scalar1=gdst_sb[:, col:col + 1], scalar2=None,
                            op0=Alu.is_equal)
                        if first:
                            agg[br] = psp.tile([128, TW], f32, tag=f"agg{br}",
                                               name=f"agg{br}")
                        nc.tensor.matmul(agg[br][:, :w], g[:, j, :], ind[:, :w],
                                         start=first, stop=last)
                        if last:
                            acc = wk.tile([128, TW], f32, tag="accT")
                            nc.vector.tensor_copy(acc[:, :w], agg[br][:, :w])
                            accTs[br] = acc
                        blk_i += 1
                finish_tile(*cur)
                assert blk_i == len(sched)
                return yT

            y1T = rgcn_layer(xf[0], x1T, is_last=False)
            nc.gpsimd.collective_compute(
                "AllGather", mybir.AluOpType.bypass,
                replica_groups=[list(range(M))],
                ins=[y_loc[:]], outs=[xf[1][:]])
            y2T = rgcn_layer(xf[1], y1T, is_last=True)

            # ================= out MLP =================
            outT = big.tile([2, L], f32, tag="outT")
            for c in range(NCH):
                w = _tile_w(c)
                ps = psp.tile([128, TW], f32, tag="out")
                nc.tensor.matmul(ps[:, :w], wout1_sb[:],
                                 y2T[:, c * TW:c * TW + w], start=True, stop=True)
                z1 = wk.tile([128, TW], f32, tag="x0c")
                leaky_from_psum(z1[:, :w], ps[:, :w], bias_sb[:, 3:4], w)
                ps2 = psp.tile([2, TW], f32, tag="out2")
                nc.tensor.matmul(ps2[:, :w], wout2_sb[:], z1[:, :w],
                                 start=True, stop=True)
                nc.vector.tensor_scalar(
                    out=outT[:, c * TW:c * TW + w], in0=ps2[:, :w],
                    scalar1=bout2_sb[:, 0:1], scalar2=None, op0=Alu.add)
            nc.sync.dma_start(out=out_d[:], in_=outT[:])

    nc.compile()
    return nc


def kernel(des, tweet, num_prop, cat_prop, edge_index, edge_type,
           W_des, b_des, W_in, b_in, W_rel, W_root, b_rgcn,
           W_out1, b_out1, W_out2, b_out2):
    import time
    from concourse.bass_utils import run_bass_kernel_spmd

    des = np.asarray(des, np.float32)
    gidx_w, gdst_w, grcp_w, sched, chunks, TTOT, NBLK = _prep_edges(
        np.asarray(edge_index), np.asarray(edge_type))

    t0 = time.time()
    nc = _build(sched, chunks, TTOT, NBLK)
    t1 = time.time()

    des_pad = np.zeros((NPAD, DDES), np.float32)
    des_pad[:N] = des
    bias = np.stack([np.asarray(b_des, np.float32),
                     np.asarray(b_in, np.float32),
                     np.asarray(b_rgcn, np.float32),
                     np.asarray(b_out1, np.float32)], axis=1)  # [128,4]
    common = {
        "wdes": np.asarray(W_des, np.float32),
        "win": np.asarray(W_in, np.float32),
        "wroot": np.asarray(W_root, np.float32),
        "wrel": np.asarray(W_rel, np.float32),
        "wout1": np.asarray(W_out1, np.float32),
        "wout2": np.asarray(W_out2, np.float32),
        "bias": bias,
        "bout2": np.asarray(b_out2, np.float32).reshape(2, 1),
    }
    in_maps = []
    for m in range(M):
        in_maps.append({
            "desT": np.ascontiguousarray(des_pad[m * L:(m + 1) * L].T),
            "gidx": gidx_w[m], "gdst": gdst_w[m], "grcp": grcp_w[m],
            **common,
        })

    trace = bool(_LAST.get("trace"))
    res = run_bass_kernel_spmd(nc, in_maps, list(range(M)), trace=trace)
    t2 = time.time()
    _LAST["build_s"] = t1 - t0
    _LAST["run_s"] = t2 - t1
    _LAST["exec_ns"] = res.exec_time_ns
    _LAST["ttot"] = TTOT

    out = np.concatenate([res.results[m]["out"].T for m in range(M)], axis=0)
    return np.ascontiguousarray(out[:N])


# revision 15
# speedup vs baseline: 1.5994x; 1.2592x over previous
"""BotRGCN forward on 8 TRN2 NeuronCores (Bass/Tile SPMD kernel).

Strategy (self-contained; shapes hardcoded for nn_BotRGCN1):
  - Nodes sharded 8-way (6272/core, N padded 50000->50176).
  - Dense MLPs node-parallel, feature-major on-chip ([128 feat, nodes]).
  - RGCN layer: aggregate-then-transform.  Edge messages are gathered
    with dma_gather (bf16 node features, 256B rows) and segment-summed
    on the TensorEngine via per-block weighted one-hot matmuls
    (Ind[tok,dst] = (iota==dst_local)*(1/deg)), accumulated in PSUM per
    (relation, dst-tile-of-512).  Transform = W_r^T @ acc on PE.
  - Boundary exchange: AllGather of bf16 activations between layers.
  - Edges are partitioned by dst core, grouped by (src-half, dst-tile,
    relation) so every dma_gather has an int16-safe base; the block
    schedule is the max over cores (SPMD: one program for all 8).
"""
import numpy as np
import ml_dtypes

N = 50000
M = 8
L = 6272            # nodes per core (N padded to 50176)
NPAD = M * L
D = 128
DDES = 768
R = 5
TW = 512            # dst tile width
NT = 13             # ceil(L/TW): 12 full + 1 of 128
SPLIT = 32768       # src half split for int16 gather indices
CHUNK = 256        # tokens per dma_gather (64 desc/engine packet limit)
SLOPE = 0.01
NQUEUES = 4

_LAST = {}          # exec stats for test harness


def _tile_w(t):
    return min(TW, L - t * TW)


def _prep_edges(edge_index, edge_type):
    """Per-core token streams + shared (max-over-cores) block schedule."""
    src = np.asarray(edge_index[0], dtype=np.int64)
    dst = np.asarray(edge_index[1], dtype=np.int64)
    et = np.asarray(edge_type, dtype=np.int64)

    core = dst // L
    dloc = dst % L

    # group tokens per core: key = (half, tile, rel)
    per_core_groups = []
    for m in range(M):
        sel = np.nonzero(core == m)[0]
        s, dl, r = src[sel], dloc[sel], et[sel]
        cnt = np.zeros((R, L), np.float32)
        np.add.at(cnt, (r, dl), 1.0)
        rcp = 1.0 / np.maximum(cnt, 1.0)
        h = (s >= SPLIT).astype(np.int64)
        t = dl // TW
        key = ((h * NT) + t) * R + r
        order = np.argsort(key, kind="stable")
        s, dl, r, h, t, key = s[order], dl[order], r[order], h[order], t[order], key[order]
        groups = {}
        bounds = np.searchsorted(key, np.arange(2 * NT * R + 1))
        for gk in range(2 * NT * R):
            a, b = bounds[gk], bounds[gk + 1]
            gh, gt, gr = gk // (NT * R), (gk // R) % NT, gk % R
            gs = s[a:b]
            gidx = np.where(gh == 0, gs, gs - SPLIT).astype(np.int16)
            gdst = (dl[a:b] - gt * TW).astype(np.float32)
            grcp = rcp[r[a:b], dl[a:b]].astype(np.float32)
            groups[(gh, gt, gr)] = (gidx, gdst, grcp)
        per_core_groups.append(groups)

    # shared schedule: blocks per group = max over cores (>=1)
    nblk = {}
    for gh in range(2):
        for gt in range(NT):
            for gr in range(R):
                mx = max(len(per_core_groups[m][(gh, gt, gr)][0]) for m in range(M))
                nblk[(gh, gt, gr)] = max(1, -(-mx // 128))

    # build padded per-core streams in fixed (h, t, r) order
    TTOT = 128 * sum(nblk.values())
    NBLK = TTOT // 128
    gidx_all = np.zeros((M, TTOT), np.int16)
    gdst_all = np.full((M, TTOT), -1.0, np.float32)
    grcp_all = np.zeros((M, TTOT), np.float32)
    pos = 0
    sched = []   # per block: (h, t, r, first, last)
    half_tok = [0, 0]
    for gh in range(2):
        for gt in range(NT):
            for gr in range(R):
                nb = nblk[(gh, gt, gr)]
                for m in range(M):
                    gi, gd, gc = per_core_groups[m][(gh, gt, gr)]
                    n = len(gi)
                    gidx_all[m, pos:pos + n] = gi
                    gdst_all[m, pos:pos + n] = gd
                    grcp_all[m, pos:pos + n] = gc
                for j in range(nb):
                    sched.append((gh, gt, gr, j == 0, j == nb - 1))
                pos += nb * 128
                half_tok[gh] += nb * 128
    assert pos == TTOT

    # gather chunks: per half, cut every CHUNK tokens
    chunks = []  # (half, tok_start, ntok)
    off = 0
    for gh in range(2):
        th = half_tok[gh]
        s0 = 0
        while s0 < th:
            n = min(CHUNK, th - s0)
            chunks.append((gh, off + s0, n))
            s0 += n
        off += th

    # wrap layouts for SBUF
    gidx_w = np.tile(
        gidx_all.reshape(M, TTOT // 16, 16).transpose(0, 2, 1), (1, 8, 1)
    ).copy()                                            # [M, 128, TTOT//16]
    gdst_w = gdst_all.reshape(M, NBLK, 128).transpose(0, 2, 1).copy()  # [M,128,NBLK]
    grcp_w = grcp_all.reshape(M, NBLK, 128).transpose(0, 2, 1).copy()
    return gidx_w, gdst_w, grcp_w, sched, chunks, TTOT, NBLK


def _build(sched, chunks, TTOT, NBLK):
    from concourse import bacc, tile, mybir

    nc = bacc.Bacc("TRN2", target_bir_lowering=False, debug=False,
                   num_devices=M, num_swdge_queues=NQUEUES)
    f32, bf16, i16 = mybir.dt.float32, mybir.dt.bfloat16, mybir.dt.int16
    f16 = mybir.dt.float16
    Alu = mybir.AluOpType

    desT_d = nc.dram_tensor("desT", [DDES, L], f32, kind="ExternalInput")
    gidx_d = nc.dram_tensor("gidx", [128, TTOT // 16], i16, kind="ExternalInput")
    gdst_d = nc.dram_tensor("gdst", [128, NBLK], f32, kind="ExternalInput")
    grcp_d = nc.dram_tensor("grcp", [128, NBLK], f32, kind="ExternalInput")
    wdes_d = nc.dram_tensor("wdes", [DDES, D], f32, kind="ExternalInput")
    win_d = nc.dram_tensor("win", [D, D], f32, kind="ExternalInput")
    wroot_d = nc.dram_tensor("wroot", [D, D], f32, kind="ExternalInput")
    wrel_d = nc.dram_tensor("wrel", [R, D, D], f32, kind="ExternalInput")
    wout1_d = nc.dram_tensor("wout1", [D, D], f32, kind="ExternalInput")
    wout2_d = nc.dram_tensor("wout2", [D, 2], f32, kind="ExternalInput")
    bias_d = nc.dram_tensor("bias", [D, 4], f32, kind="ExternalInput")  # des,in,rgcn,out1
    bout2_d = nc.dram_tensor("bout2", [2, 1], f32, kind="ExternalInput")
    out_d = nc.dram_tensor("out", [2, L], f32, kind="ExternalOutput")

    y_loc = nc.dram_tensor("y_loc", [L, D], f16)
    xf = [nc.dram_tensor(f"xf{i}", [NPAD, D], f16, addr_space="Shared")
          for i in range(2)]

    iota = nc.inline_tensor(
        np.broadcast_to(np.arange(TW, dtype=np.float16), (128, TW)).copy(), "iota")
    ident = nc.inline_tensor(np.eye(128, dtype=np.float32), "ident")

    NCH = -(-L // TW)  # mlp col chunks (= NT)

    with tile.TileContext(nc) as tc:
        with (
            tc.tile_pool(name="cst", bufs=1) as cst,
            tc.tile_pool(name="big", bufs=2) as big,
            tc.tile_pool(name="wk", bufs=4) as wk,
            tc.tile_pool(name="ps", bufs=1, space="PSUM") as psp,
        ):
            # ---- constants to SBUF ----
            iota_sb = cst.tile([128, TW], f16)
            nc.sync.dma_start(out=iota_sb[:], in_=iota[:])
            ident_sb = cst.tile([128, 128], f32)
            nc.sync.dma_start(out=ident_sb[:], in_=ident[:])
            gidx_sb = cst.tile([128, TTOT // 16], i16)
            nc.sync.dma_start(out=gidx_sb[:], in_=gidx_d[:])
            gdst_sb = cst.tile([128, NBLK], f32)
            nc.sync.dma_start(out=gdst_sb[:], in_=gdst_d[:])
            grcp_sb = cst.tile([128, NBLK], f32)
            nc.sync.dma_start(out=grcp_sb[:], in_=grcp_d[:])
            wdes_sb = cst.tile([128, 6, D], f32)
            for k in range(6):
                nc.sync.dma_start(out=wdes_sb[:, k, :], in_=wdes_d[k * 128:(k + 1) * 128, :])
            win_sb = cst.tile([128, D], f32)
            nc.sync.dma_start(out=win_sb[:], in_=win_d[:])
            wroot_sb = cst.tile([128, D], f32)
            nc.sync.dma_start(out=wroot_sb[:], in_=wroot_d[:])
            wrel_sb = cst.tile([128, R, D], f32)
            for r in range(R):
                nc.sync.dma_start(out=wrel_sb[:, r, :], in_=wrel_d[r])
            wout1_sb = cst.tile([128, D], f32)
            nc.sync.dma_start(out=wout1_sb[:], in_=wout1_d[:])
            wout2_sb = cst.tile([128, 2], f32)
            nc.sync.dma_start(out=wout2_sb[:], in_=wout2_d[:])
            bias_sb = cst.tile([128, 4], f32)
            nc.sync.dma_start(out=bias_sb[:], in_=bias_d[:])
            bout2_sb = cst.tile([2, 1], f32)
            nc.sync.dma_start(out=bout2_sb[:], in_=bout2_d[:])

            def leaky_from_psum(dst_f32, ps_ap, bias_col, w):
                """dst = leaky(ps + bias), feature-major [128, w]."""
                t1 = wk.tile([128, TW], f32, tag="lk1")
                nc.vector.tensor_scalar(
                    out=t1[:, :w], in0=ps_ap, scalar1=bias_col, scalar2=None,
                    op0=Alu.add)
                t2 = wk.tile([128, TW], f32, tag="lk2")
                nc.vector.tensor_scalar_mul(t2[:, :w], t1[:, :w], SLOPE)
                nc.vector.tensor_tensor(
                    out=dst_f32, in0=t1[:, :w], in1=t2[:, :w], op=Alu.max)

            def transpose_store(src_f32_ap, t, w):
                """feature-major [128, w] slice of tile t -> y_loc + (later AG)."""
                for b in range(-(-w // 128)):
                    bw = min(128, w - b * 128)
                    trp = psp.tile([128, 128], f32, tag="tr")
                    nc.tensor.transpose(
                        trp[:bw, :], src_f32_ap[:, b * 128:b * 128 + bw], ident_sb[:])
                    ynm = wk.tile([128, D], f16, tag="ynm")
                    nc.vector.tensor_copy(ynm[:bw, :], trp[:bw, :])
                    r0 = t * TW + b * 128
                    nc.sync.dma_start(out=y_loc[r0:r0 + bw, :], in_=ynm[:bw, :])

            # ================= MLP =================
            x1T = big.tile([128, L], f32, tag="bigT")
            for c in range(NCH):
                w = _tile_w(c)
                ps = psp.tile([128, TW], f32, tag="out")
                for k in range(6):
                    dt = wk.tile([128, TW], f32, tag="des")
                    nc.sync.dma_start(
                        out=dt[:, :w],
                        in_=desT_d[k * 128:(k + 1) * 128, c * TW:c * TW + w])
                    nc.tensor.matmul(ps[:, :w], wdes_sb[:, k, :], dt[:, :w],
                                     start=(k == 0), stop=(k == 5))
                x0c = wk.tile([128, TW], f32, tag="x0c")
                leaky_from_psum(x0c[:, :w], ps[:, :w], bias_sb[:, 0:1], w)
                ps2 = psp.tile([128, TW], f32, tag="out")
                nc.tensor.matmul(ps2[:, :w], win_sb[:], x0c[:, :w],
                                 start=True, stop=True)
                leaky_from_psum(x1T[:, c * TW:c * TW + w], ps2[:, :w],
                                bias_sb[:, 1:2], w)
                transpose_store(x1T[:, c * TW:c * TW + w], c, w)

            nc.gpsimd.collective_compute(
                "AllGather", mybir.AluOpType.bypass,
                replica_groups=[list(range(M))],
                ins=[y_loc[:]], outs=[xf[0][:]])

            # ================= RGCN layers =================
            # queue must track Tile's global DMASW lane rotation: gathers are
            # the ONLY Pool-engine DMA insts, so a single global counter keeps
            # lane%4 == queue for every gather.
            qctr = [0]

            def rgcn_layer(xfull, x_curT, is_last):
                yT = big.tile([128, L], f32, tag="bigT")
                half_base = [xfull[0:SPLIT, :], xfull[SPLIT:NPAD, :]]
                agg = {}       # r -> psum tile for current (h,t)
                accTs = {}     # r -> drained SBUF acc for current (h,t)

                def finish_tile(h, t):
                    w = _tile_w(t)
                    ops = psp.tile([128, TW], f32, tag="out")
                    if h == 0:
                        nc.tensor.matmul(ops[:, :w], wroot_sb[:],
                                         x_curT[:, t * TW:t * TW + w],
                                         start=True, stop=False)
                    for ri in range(R):
                        nc.tensor.matmul(ops[:, :w], wrel_sb[:, ri, :],
                                         accTs[ri][:, :w],
                                         start=(h == 1 and ri == 0),
                                         stop=(ri == R - 1))
                    if h == 0:
                        nc.vector.tensor_scalar(
                            out=yT[:, t * TW:t * TW + w], in0=ops[:, :w],
                            scalar1=bias_sb[:, 2:3], scalar2=None, op0=Alu.add)
                    else:
                        nc.vector.tensor_tensor(
                            out=yT[:, t * TW:t * TW + w], in0=ops[:, :w],
                            in1=yT[:, t * TW:t * TW + w], op=Alu.add)
                        if not is_last:
                            transpose_store(yT[:, t * TW:t * TW + w], t, w)
                    accTs.clear()

                blk_i = 0
                cur = None  # (h, t)
                for (h, s0, ntok) in chunks:
                    nb = ntok // 128
                    g = wk.tile([128, CHUNK // 128, D], f16, tag="g", bufs=8)
                    nc.gpsimd.dma_gather(
                        out_ap=g[:, :nb, :],
                        in_ap=half_base[h],
                        idxs_ap=gidx_sb[:, s0 // 16:(s0 + ntok) // 16],
                        num_idxs=ntok,
                        num_idxs_reg=ntok,
                        elem_size=D,
                        queue_num=qctr[0] % NQUEUES,
                    )
                    qctr[0] += 1
                    for j in range(nb):
                        bh, bt, br, first, last = sched[blk_i]
                        assert bh == h
                        if cur is None:
                            cur = (bh, bt)
                        elif cur != (bh, bt):
                            finish_tile(*cur)
                            cur = (bh, bt)
                        w = _tile_w(bt)
                        col = s0 // 128 + j
                        nc.scalar.activation(
                            g[:, j, :], g[:, j, :],
                            mybir.ActivationFunctionType.Copy,
                            scale=grcp_sb[:, col:col + 1])
                        ind = wk.tile([128, TW], f16, tag="ind", bufs=8)
                        nc.vector.tensor_scalar(
                            out=ind[:, :w], in0=iota_sb[:, :w],
                            scalar1=gdst_sb[:, col:col + 1], scalar2=None,
                            op0=Alu.is_equal)
                        if first:
                            agg[br] = psp.tile([128, TW], f32, tag="agg",
                                               name=f"agg{br}", bufs=3)
                        nc.tensor.matmul(agg[br][:, :w], g[:, j, :], ind[:, :w],
                                         start=first, stop=last)
                        if last:
                            acc = wk.tile([128, TW], f32, tag="accT", bufs=6)
                            nc.scalar.activation(
                                acc[:, :w], agg[br][:, :w],
                                mybir.ActivationFunctionType.Copy)
                            accTs[br] = acc
                        blk_i += 1
                finish_tile(*cur)
                assert blk_i == len(sched)
                return yT

            y1T = rgcn_layer(xf[0], x1T, is_last=False)
            nc.gpsimd.collective_compute(
                "AllGather", mybir.AluOpType.bypass,
                replica_groups=[list(range(M))],
                ins=[y_loc[:]], outs=[xf[1][:]])
            y2T = rgcn_layer(xf[1], y1T, is_last=True)

            # ================= out MLP =================
            outT = big.tile([2, L], f32, tag="outT")
            for c in range(NCH):
                w = _tile_w(c)
                ps = psp.tile([128, TW], f32, tag="out")
                nc.tensor.matmul(ps[:, :w], wout1_sb[:],
                                 y2T[:, c * TW:c * TW + w], start=True, stop=True)
                z1 = wk.tile([128, TW], f32, tag="x0c")
                leaky_from_psum(z1[:, :w], ps[:, :w], bias_sb[:, 3:4], w)
                ps2 = psp.tile([2, TW], f32, tag="out2")
                nc.tensor.matmul(ps2[:, :w], wout2_sb[:], z1[:, :w],
                                 start=True, stop=True)
                nc.vector.tensor_scalar(
                    out=outT[:, c * TW:c * TW + w], in0=ps2[:, :w],
                    scalar1=bout2_sb[:, 0:1], scalar2=None, op0=Alu.add)
            nc.sync.dma_start(out=out_d[:], in_=outT[:])

    nc.compile()
    return nc


def kernel(des, tweet, num_prop, cat_prop, edge_index, edge_type,
           W_des, b_des, W_in, b_in, W_rel, W_root, b_rgcn,
           W_out1, b_out1, W_out2, b_out2):
    import time
    from concourse.bass_utils import run_bass_kernel_spmd

    des = np.asarray(des, np.float32)
    gidx_w, gdst_w, grcp_w, sched, chunks, TTOT, NBLK = _prep_edges(
        np.asarray(edge_index), np.asarray(edge_type))

    t0 = time.time()
    nc = _build(sched, chunks, TTOT, NBLK)
    t1 = time.time()

    des_pad = np.zeros((NPAD, DDES), np.float32)
    des_pad[:N] = des
    bias = np.stack([np.asarray(b_des, np.float32),
                     np.asarray(b_in, np.float32),
                     np.asarray(b_rgcn, np.float32),
                     np.asarray(b_out1, np.float32)], axis=1)  # [128,4]
    common = {
        "wdes": np.asarray(W_des, np.float32),
        "win": np.asarray(W_in, np.float32),
        "wroot": np.asarray(W_root, np.float32),
        "wrel": np.asarray(W_rel, np.float32),
        "wout1": np.asarray(W_out1, np.float32),
        "wout2": np.asarray(W_out2, np.float32),
        "bias": bias,
        "bout2": np.asarray(b_out2, np.float32).reshape(2, 1),
    }
    in_maps = []
    for m in range(M):
        in_maps.append({
            "desT": np.ascontiguousarray(des_pad[m * L:(m + 1) * L].T),
            "gidx": gidx_w[m], "gdst": gdst_w[m], "grcp": grcp_w[m],
            **common,
        })

    trace = bool(_LAST.get("trace"))
    res = run_bass_kernel_spmd(nc, in_maps, list(range(M)), trace=trace)
    t2 = time.time()
    _LAST["build_s"] = t1 - t0
    _LAST["run_s"] = t2 - t1
    _LAST["exec_ns"] = res.exec_time_ns
    _LAST["ttot"] = TTOT

    out = np.concatenate([res.results[m]["out"].T for m in range(M)], axis=0)
    return np.ascontiguousarray(out[:N])
